# revision 7
# baseline (speedup 1.0000x reference)
"""Causal attention (B=4, S=2048, D=1024) on 8 Trainium2 NeuronCores.

Sharding: data-parallel over batch (4) x query-block-parallel (2 cores per
batch).  Global q-tiles (128 rows each, 16 per batch) are dealt round-robin:
core h=0 of a pair takes even tiles, h=1 odd tiles.  The program rounds every
q-tile's causal key-extent up to a multiple of 256 -- tile pair (2j, 2j+1)
then shares the extent 256*(j+1), so both cores run the *same* instruction
stream (SPMD) and the residual causal masking is supplied as a per-core
additive-mask input.

Reassociated algebra (cuts per-core matmul work 15.5 -> 11.1 GFLOP):
  scores = (x Wq)(x Wk)^T / 32 = x A x^T   with A = (Wq/32) Wk^T
  P V    = P (x Wv) = (P x) Wv
so the K/V projections over the full (pair-duplicated) sequence are replaced
by the once-per-core A (d x d) and per-query-block (P x) Wv products:
  A      [d',d]  = sum_e wk[d',e] wq[d,e]/32          (128 MMs @ N=512)
  R^T    [d,q]   = sum_d' A[d',d] x[q,d']             (128 MMs, strided q cols)
  S      [q,k]   = sum_d R^T[d,q]^T x[k,d]            (144 MMs)
  P      = exp(S + mask), row sums via activation accum_out
  Px     [q,d]   = sum_k P^T[k,q]^T x[k,d]            (144 MMs)
  O      [q,e]   = sum_d Px^T[d,q]^T wv[d,e] / rowsum (128 MMs)

All transposes (x^T, P^T, Px^T) run on the DMA engines via the XBAR
DMA-transpose (InstDmaTransposeAnt, ~14ns per 16x128 tile) -- the PE does
only matmuls.  XBARs are kept on the SP queue and plain DMAs on the ACT
queue (DMATranspose<->DMACopy on one queue serializes on mode transition).
The attention j-loop is software-pipelined with a 2-stage skew
(S(j) | Px(j-1) | O(j-2)) so the PE never waits on the exp->XBAR->matmul
dependency chains.  Weights arrive as pre-transposed bf16 ([e,d] for wq/wk)
from the host; wq is pre-scaled by 1/32 (exact power of 2).
"""

import os

os.environ.setdefault("MYCRO_LOCAL_CACHE", "1")

import ml_dtypes
import numpy as np

import concourse.bacc as bacc
import concourse.tile as tile
from concourse import mybir
from concourse.bass_utils import run_bass_kernel_spmd

B, S, D = 4, 2048, 1024
P = 128
QL = S // 2          # queries per core
NCORES = 8
DT = D // P          # 8 d-tiles
ST = S // P          # 16 s-tiles
NQT = QL // P        # 8 q-tiles per core
F32 = mybir.dt.float32
BF16 = mybir.dt.bfloat16
NEG = -30000.0       # additive mask value; exp() underflows to exactly 0


def _chunks(extent):
    out, o = [], 0
    while o < extent:
        w = min(512, extent - o)
        out.append((o, w))
        o += w
    return out


def _body(tc, x, xq, wqt, wkt, wv, mask, out):
    nc = tc.nc
    with (
        tc.tile_pool(name="consts", bufs=1) as consts,
        tc.tile_pool(name="main", bufs=1) as main,
        tc.tile_pool(name="pmm", bufs=4, space="PSUM") as pmm,
        tc.tile_pool(name="psO", bufs=2, space="PSUM") as psO,
    ):
        mask_sb = consts.tile([P, 256], F32)
        nc.scalar.dma_start(mask_sb, mask)

        xT = main.tile([P, DT, S], BF16)     # [d_in, d_tile, s]
        xqT = main.tile([P, DT, QL], BF16)   # [d_in, d_tile, q]
        xn = main.tile([P, ST, D], BF16)     # [s_in, s_tile, d]
        qa = main.tile([P, DT, QL], BF16)    # R^T = (xq A)^T : [d_in, d_tile, q]
        wv_sb = main.tile([P, DT, D], BF16)  # [d_in, d_tile, e]

        # x^T via XBAR (SP queue carries ONLY DMA-transposes)
        for d in range(DT):
            nc.sync.dma_start(xqT[:, d, :], xq[:, d * P:(d + 1) * P],
                              transpose=True)
        for d in range(DT):
            nc.sync.dma_start(xT[:, d, :], x[:, d * P:(d + 1) * P],
                              transpose=True)

        with tc.tile_pool(name="wscope", bufs=1) as ws:
            wkt_sb = ws.tile([P, DT, D], BF16)   # [e_in, e_tile, d']
            wqt_sb = ws.tile([P, DT, D], BF16)   # [e_in, e_tile, d]
            a_sb = ws.tile([P, DT, D], BF16)     # A: [d'_in, d'_tile, d]

            # weight DMAs ordered so A's first matmul group waits on ~1.25MB:
            # wq^T column-block 0 + wk^T first halves, then the rest streams.
            def wqt_col(dcb):
                for et in range(DT):
                    nc.scalar.dma_start(
                        wqt_sb[:, et, dcb * P:(dcb + 1) * P],
                        wqt[et * P:(et + 1) * P, dcb * P:(dcb + 1) * P])

            def wkt_half(ch):
                for et in range(DT):
                    nc.scalar.dma_start(
                        wkt_sb[:, et, ch * 512:(ch + 1) * 512],
                        wkt[et * P:(et + 1) * P, ch * 512:(ch + 1) * 512])

            wqt_col(0)
            wkt_half(0)
            for dcb in range(1, DT):
                wqt_col(dcb)
            wkt_half(1)
            for s in range(ST):
                nc.scalar.dma_start(xn[:, s, :], x[s * P:(s + 1) * P, :])
            for d in range(DT):
                nc.scalar.dma_start(wv_sb[:, d, :], wv[d * P:(d + 1) * P, :])

            # ---- A[d'tile, d] = sum_e wq[d',e]/32 wk[d,e]
            for ch in range(2):
                for dtp in range(DT):
                    ps = pmm.tile([P, 512], F32, tag="mm")
                    for et in range(DT):
                        nc.tensor.matmul(
                            ps, wqt_sb[:, et, dtp * P:(dtp + 1) * P],
                            wkt_sb[:, et, ch * 512:(ch + 1) * 512],
                            start=(et == 0), stop=(et == DT - 1))
                    nc.scalar.copy(a_sb[:, dtp, ch * 512:(ch + 1) * 512], ps)

            # ---- R^T[dtile, q] = sum_d' A[d', d] xq[q, d']
            for qc in (1, 0):        # qc=1 first: scores start at j=7
                for dt in range(DT):
                    ps = pmm.tile([P, 512], F32, tag="mm")
                    for dtp in range(DT):
                        nc.tensor.matmul(
                            ps, a_sb[:, dtp, dt * P:(dt + 1) * P],
                            xqT[:, dtp, qc * 512:(qc + 1) * 512],
                            start=(dtp == 0), stop=(dtp == DT - 1))
                    nc.scalar.copy(qa[:, dt, qc * 512:(qc + 1) * 512], ps)

        # ------------------------------ attention --------------------------
        with (
            tc.tile_pool(name="pp", bufs=2) as pp,
            tc.tile_pool(name="ptp", bufs=2) as ptp,
            tc.tile_pool(name="pxp", bufs=2) as pxp,
            tc.tile_pool(name="pxtp", bufs=2) as pxtp,
            tc.tile_pool(name="op", bufs=2) as op,
            tc.tile_pool(name="stats", bufs=3) as spool,
        ):
            st = {}

            def emit_scores(j):
                ext = 256 * (j + 1)
                p_sb = pp.tile([P, S], BF16, tag="p")
                pt = ptp.tile([P, ST, P], BF16, tag="pt")
                lsum = spool.tile([P, ST], F32, tag="lsum")
                for (o, w) in _chunks(ext):
                    ps = pmm.tile([P, 512], F32, tag="mm")
                    for dt in range(DT):
                        nc.tensor.matmul(
                            ps[:, :w], qa[:, dt, j * P:(j + 1) * P],
                            xT[:, dt, o:o + w],
                            start=(dt == 0), stop=(dt == DT - 1))
                    if o + w == ext:
                        nc.vector.tensor_add(
                            ps[:, w - 256:w], ps[:, w - 256:w], mask_sb)
                    for si in range(w // P):
                        col = o + si * P
                        kt = col // P
                        nc.scalar.activation(
                            p_sb[:, col:col + P], ps[:, si * P:(si + 1) * P],
                            mybir.ActivationFunctionType.Exp,
                            accum_out=lsum[:, kt:kt + 1])
                        nc.sync.dma_start(pt[:, kt, :], p_sb[:, col:col + P],
                                          transpose=True)
                l_ = spool.tile([P, 1], F32, tag="l")
                nc.vector.reduce_sum(l_, lsum[:, :ext // P],
                                     axis=mybir.AxisListType.X)
                linv = spool.tile([P, 1], F32, tag="linv")
                nc.vector.reciprocal(linv, l_)
                st[j] = {"pt": pt, "linv": linv}

            def emit_px(j):
                nk = 2 * (j + 1)
                pt = st[j]["pt"]
                px_sb = pxp.tile([P, D], BF16, tag="px")
                pxt = pxtp.tile([P, DT, P], BF16, tag="pxt")
                for ec in range(2):
                    ps = pmm.tile([P, 512], F32, tag="mm")
                    for kt in range(nk):
                        nc.tensor.matmul(
                            ps, pt[:, kt, :], xn[:, kt, ec * 512:(ec + 1) * 512],
                            start=(kt == 0), stop=(kt == nk - 1))
                    nc.scalar.copy(px_sb[:, ec * 512:(ec + 1) * 512], ps)
                for dt in range(DT):
                    nc.sync.dma_start(pxt[:, dt, :],
                                      px_sb[:, dt * P:(dt + 1) * P],
                                      transpose=True)
                st[j]["pxt"] = pxt

            def emit_o(j):
                pxt = st[j]["pxt"]
                linv = st[j]["linv"]
                po = psO.tile([P, D], F32, tag="o")
                for ec in range(2):
                    for dt in range(DT):
                        nc.tensor.matmul(
                            po[:, ec * 512:(ec + 1) * 512], pxt[:, dt, :],
                            wv_sb[:, dt, ec * 512:(ec + 1) * 512],
                            start=(dt == 0), stop=(dt == DT - 1))
                o_sb = op.tile([P, D], F32, tag="osb")
                for ec in range(2):
                    nc.vector.tensor_scalar_mul(
                        o_sb[:, ec * 512:(ec + 1) * 512],
                        po[:, ec * 512:(ec + 1) * 512], linv)
                nc.scalar.dma_start(out[j * P:(j + 1) * P, :], o_sb)
                del st[j]

            js = [7, 6, 5, 4, 3, 2, 1, 0]
            for step in range(len(js) + 2):
                if step < len(js):
                    emit_scores(js[step])
                if 1 <= step <= len(js):
                    emit_px(js[step - 1])
                if step >= 2:
                    emit_o(js[step - 2])


_PROG = None


def _get_prog():
    global _PROG
    if _PROG is None:
        nc = bacc.Bacc("TRN2", target_bir_lowering=False, debug=False,
                       enable_asserts=False)
        x = nc.dram_tensor("x", (S, D), BF16, kind="ExternalInput").ap()
        xq = nc.dram_tensor("xq", (QL, D), BF16, kind="ExternalInput").ap()
        wqt = nc.dram_tensor("wqt", (D, D), BF16, kind="ExternalInput").ap()
        wkt = nc.dram_tensor("wkt", (D, D), BF16, kind="ExternalInput").ap()
        wv = nc.dram_tensor("wv", (D, D), BF16, kind="ExternalInput").ap()
        mask = nc.dram_tensor("mask", (P, 256), F32, kind="ExternalInput").ap()
        out = nc.dram_tensor("out", (QL, D), F32, kind="ExternalOutput").ap()
        with tile.TileContext(nc) as tc:
            _body(tc, x, xq, wqt, wkt, wv, mask, out)
        nc.compile()
        _PROG = nc
    return _PROG


def _mask_np(h):
    r = np.arange(P)[:, None]
    c = np.arange(P)[None, :]
    tri = np.where(c <= r, 0.0, NEG).astype(np.float32)
    m = np.zeros((P, 256), np.float32)
    if h == 0:
        m[:, :P] = tri
        m[:, P:] = NEG
    else:
        m[:, P:] = tri
    return m


def _prep_shared(inputs):
    bf = ml_dtypes.bfloat16
    wq = np.asarray(inputs["wq"], np.float32)
    wk = np.asarray(inputs["wk"], np.float32)
    wv = np.asarray(inputs["wv"], np.float32)
    return {
        "wqt": np.ascontiguousarray((wq / 32.0).T.astype(bf)),
        "wkt": np.ascontiguousarray(wk.T.astype(bf)),
        "wv": np.ascontiguousarray(wv.astype(bf)),
    }


def _in_map_for_core(inputs, core, shared=None):
    b, h = core // 2, core % 2
    if shared is None:
        shared = _prep_shared(inputs)
    xb = np.ascontiguousarray(
        np.asarray(inputs["x"], np.float32)[b].astype(ml_dtypes.bfloat16))
    xqb = np.ascontiguousarray(xb.reshape(NQT, 2, P, D)[:, h].reshape(QL, D))
    return {"x": xb, "xq": xqb, "mask": _mask_np(h), **shared}


def _run(inputs, trace=False, tmpdir=None):
    nc = _get_prog()
    shared = _prep_shared(inputs)
    in_maps = [_in_map_for_core(inputs, c, shared) for c in range(NCORES)]
    try:
        res = run_bass_kernel_spmd(nc, in_maps, core_ids=list(range(NCORES)),
                                   trace=trace, tmpdir=tmpdir)
    except Exception:
        # first execution of a fresh NEFF occasionally trips a transient
        # device error on this stack; one retry has always succeeded
        res = run_bass_kernel_spmd(nc, in_maps, core_ids=list(range(NCORES)),
                                   trace=trace, tmpdir=tmpdir)
    outf = np.empty((B, S, D), np.float32)
    for core in range(NCORES):
        b, h = core // 2, core % 2
        o = np.asarray(res.results[core]["out"], np.float32)
        outf[b].reshape(NQT, 2, P, D)[:, h] = o.reshape(NQT, P, D)
    return outf, res


def kernel(x, wq, wk, wv):
    outf, _ = _run({"x": x, "wq": wq, "wk": wk, "wv": wv}, trace=False)
    return outf


# revision 9
# speedup vs baseline: 2.2288x; 2.2288x over previous
"""Causal attention (B=4, S=2048, D=1024) on 8 Trainium2 NeuronCores.

Sharding: data-parallel over batch (4) x query-block-parallel (2 cores per
batch).  Global q-tiles (128 rows each, 16 per batch) are dealt round-robin:
core h=0 of a pair takes even tiles, h=1 odd tiles.  The program rounds every
q-tile's causal key-extent up to a multiple of 256 -- tile pair (2j, 2j+1)
then shares the extent 256*(j+1), so both cores run the *same* instruction
stream (SPMD) and the residual causal masking is supplied as a per-core
additive-mask input.

Reassociated algebra (cuts per-core matmul work 15.5 -> 11.1 GFLOP):
  scores = (x Wq)(x Wk)^T / 32 = x A x^T   with A = (Wq/32) Wk^T
  P V    = P (x Wv) = (P x) Wv
so the K/V projections over the full (pair-duplicated) sequence are replaced
by the once-per-core A (d x d) and per-query-block (P x) Wv products:
  A      [d',d]  = sum_e wq[d',e]/32 wk[d,e]          (128 MMs @ N=512)
  R^T    [d,q]   = sum_d' A[d',d] xq[q,d']            (128 MMs)
  S      [q,k]   = sum_d R^T[d,q]^T x[k,d]            (144 MMs)
  P      = exp(S + mask), row sums via activation accum_out
  Px     [q,d]   = sum_k P^T[k,q]^T x[k,d]            (144 MMs)
  O      [q,e]   = sum_d Px^T[d,q]^T wv[d,e] / rowsum (128 MMs)

All transposes (wq^T, wk^T, x^T, xq^T, P^T, Px^T) run on the DMA engines via
the XBAR DMA-transpose, one BATCHED instruction per tensor (a 3D SBUF
destination [128, kt, n] extends the logical partition dim, so e.g. the full
[2048,1024] x transposes in a single instruction).  Per-instruction XBAR
overhead is ~1.2us, so batching matters far more than per-tile cost.  XBARs
live alone on the SP HWDGE queue; bulk loads go on the ACT HWDGE queue
before the attention phase; output stores use the gpsimd SWDGE path -- the
DMATranspose<->DMACopy xbar_mode transition serializes a queue (HW bug), so
copies and transposes are kept on disjoint paths while attention runs.
The attention j-loop is software-pipelined with a 2-stage skew
(S(j) | Px(j-1) | O(j-2)) so the PE never waits on exp->XBAR->matmul chains.
PSUM-evictions run on the DVE; exp runs per 512-chunk on ACT.
"""

import os

os.environ.setdefault("MYCRO_LOCAL_CACHE", "1")

import ml_dtypes
import numpy as np

import concourse.bacc as bacc
import concourse.tile as tile
from concourse import mybir
from concourse.bass_utils import run_bass_kernel_spmd

B, S, D = 4, 2048, 1024
P = 128
QL = S // 2          # queries per core
NCORES = 8
DT = D // P          # 8 d-tiles
ST = S // P          # 16 s-tiles
NQT = QL // P        # 8 q-tiles per core
F32 = mybir.dt.float32
BF16 = mybir.dt.bfloat16
NEG = -30000.0       # additive mask value; exp() underflows to exactly 0


def _chunks(extent):
    out, o = [], 0
    while o < extent:
        w = min(512, extent - o)
        out.append((o, w))
        o += w
    return out


def _body(tc, x, xq, wq, wk, wv, mask, out):
    nc = tc.nc
    with (
        tc.tile_pool(name="consts", bufs=1) as consts,
        tc.tile_pool(name="main", bufs=1) as main,
        tc.tile_pool(name="pmm", bufs=4, space="PSUM") as pmm,
        tc.tile_pool(name="psO", bufs=2, space="PSUM") as psO,
    ):
        mask_sb = consts.tile([P, 256], F32)

        xT = main.tile([P, DT, S], BF16)     # [d_in, d_tile, s]
        xqT = main.tile([P, DT, QL], BF16)   # [d_in, d_tile, q]
        xn = main.tile([P, ST, D], BF16)     # [s_in, s_tile, d]
        qa = main.tile([P, DT, QL], BF16)    # R^T = (xq A)^T : [d_in, d_tile, q]
        wv_sb = main.tile([P, DT, D], BF16)  # [d_in, d_tile, e]

        with tc.tile_pool(name="wscope", bufs=1) as ws:
            wqt_sb = ws.tile([P, DT, D], BF16)   # [e_in, e_tile, d']
            wkt_sb = ws.tile([P, DT, D], BF16)   # [e_in, e_tile, d]
            a_sb = ws.tile([P, DT, D], BF16)     # A: [d'_in, d'_tile, d]

            # ---- XBAR transposes, one instruction each (SP queue only)
            nc.sync.dma_start(wqt_sb[:, :, :], wq[:, :], transpose=True)
            nc.sync.dma_start(wkt_sb[:, :, :], wk[:, :], transpose=True)
            nc.sync.dma_start(xqT[:, :, :], xq[:, :], transpose=True)
            nc.sync.dma_start(xT[:, :, :], x[:, :], transpose=True)

            # ---- bulk loads (ACT queue; all complete before attention)
            nc.scalar.dma_start(mask_sb, mask)
            for g in range(4):
                nc.scalar.dma_start(
                    xn[:, 4 * g:4 * g + 4, :],
                    x[g * 512:(g + 1) * 512, :].rearrange(
                        "(st ss) d -> ss st d", ss=P))
            for g in range(2):
                nc.scalar.dma_start(
                    wv_sb[:, 4 * g:4 * g + 4, :],
                    wv[g * 512:(g + 1) * 512, :].rearrange(
                        "(t dd) e -> dd t e", dd=P))

            # ---- A[d'tile, d] = sum_e wq[d',e]/32 wk[d,e]
            for ch in range(2):
                for dtp in range(DT):
                    ps = pmm.tile([P, 512], F32, tag="mm")
                    for et in range(DT):
                        nc.tensor.matmul(
                            ps, wqt_sb[:, et, dtp * P:(dtp + 1) * P],
                            wkt_sb[:, et, ch * 512:(ch + 1) * 512],
                            start=(et == 0), stop=(et == DT - 1))
                    nc.vector.tensor_copy(
                        a_sb[:, dtp, ch * 512:(ch + 1) * 512], ps)

            # ---- R^T[dtile, q] = sum_d' A[d', d] xq[q, d']
            for qc in (1, 0):        # qc=1 first: scores start at j=7
                for dt in range(DT):
                    ps = pmm.tile([P, 512], F32, tag="mm")
                    for dtp in range(DT):
                        nc.tensor.matmul(
                            ps, a_sb[:, dtp, dt * P:(dt + 1) * P],
                            xqT[:, dtp, qc * 512:(qc + 1) * 512],
                            start=(dtp == 0), stop=(dtp == DT - 1))
                    nc.vector.tensor_copy(
                        qa[:, dt, qc * 512:(qc + 1) * 512], ps)

        # ------------------------------ attention --------------------------
        with (
            tc.tile_pool(name="pp", bufs=2) as pp,
            tc.tile_pool(name="ptp", bufs=2) as ptp,
            tc.tile_pool(name="pxp", bufs=2) as pxp,
            tc.tile_pool(name="pxtp", bufs=2) as pxtp,
            tc.tile_pool(name="op", bufs=2) as op,
            tc.tile_pool(name="stats", bufs=3) as spool,
        ):
            st = {}

            def emit_scores(j):
                ext = 256 * (j + 1)
                nchunk = len(_chunks(ext))
                p_sb = pp.tile([P, S], BF16, tag="p")
                pt = ptp.tile([P, ST, P], BF16, tag="pt")
                lsum = spool.tile([P, 4], F32, tag="lsum")
                for ci, (o, w) in enumerate(_chunks(ext)):
                    ps = pmm.tile([P, 512], F32, tag="mm")
                    for dt in range(DT):
                        nc.tensor.matmul(
                            ps[:, :w], qa[:, dt, j * P:(j + 1) * P],
                            xT[:, dt, o:o + w],
                            start=(dt == 0), stop=(dt == DT - 1))
                    if o + w == ext:
                        nc.vector.tensor_add(
                            ps[:, w - 256:w], ps[:, w - 256:w], mask_sb)
                    nc.scalar.activation(
                        p_sb[:, o:o + w], ps[:, :w],
                        mybir.ActivationFunctionType.Exp,
                        accum_out=lsum[:, ci:ci + 1])
                nc.sync.dma_start(pt[:, :ext // P, :], p_sb[:, :ext],
                                  transpose=True)
                l_ = spool.tile([P, 1], F32, tag="l")
                nc.vector.reduce_sum(l_, lsum[:, :nchunk],
                                     axis=mybir.AxisListType.X)
                linv = spool.tile([P, 1], F32, tag="linv")
                nc.vector.reciprocal(linv, l_)
                st[j] = {"pt": pt, "linv": linv}

            def emit_px(j):
                nk = 2 * (j + 1)
                pt = st[j]["pt"]
                px_sb = pxp.tile([P, D], BF16, tag="px")
                pxt = pxtp.tile([P, DT, P], BF16, tag="pxt")
                for ec in range(2):
                    ps = pmm.tile([P, 512], F32, tag="mm")
                    for kt in range(nk):
                        nc.tensor.matmul(
                            ps, pt[:, kt, :], xn[:, kt, ec * 512:(ec + 1) * 512],
                            start=(kt == 0), stop=(kt == nk - 1))
                    nc.vector.tensor_copy(px_sb[:, ec * 512:(ec + 1) * 512], ps)
                nc.sync.dma_start(pxt[:, :, :], px_sb[:, :], transpose=True)
                st[j]["pxt"] = pxt

            def emit_o(j):
                pxt = st[j]["pxt"]
                linv = st[j]["linv"]
                po = psO.tile([P, D], F32, tag="o")
                for ec in range(2):
                    for dt in range(DT):
                        nc.tensor.matmul(
                            po[:, ec * 512:(ec + 1) * 512], pxt[:, dt, :],
                            wv_sb[:, dt, ec * 512:(ec + 1) * 512],
                            start=(dt == 0), stop=(dt == DT - 1))
                o_sb = op.tile([P, D], F32, tag="osb")
                for ec in range(2):
                    nc.vector.tensor_scalar_mul(
                        o_sb[:, ec * 512:(ec + 1) * 512],
                        po[:, ec * 512:(ec + 1) * 512], linv)
                nc.gpsimd.dma_start(out[j * P:(j + 1) * P, :], o_sb)
                del st[j]

            js = [7, 6, 5, 4, 3, 2, 1, 0]
            for step in range(len(js) + 2):
                if step < len(js):
                    emit_scores(js[step])
                if 1 <= step <= len(js):
                    emit_px(js[step - 1])
                if step >= 2:
                    emit_o(js[step - 2])


_PROG = None


def _get_prog():
    global _PROG
    if _PROG is None:
        nc = bacc.Bacc("TRN2", target_bir_lowering=False, debug=False,
                       enable_asserts=False)
        x = nc.dram_tensor("x", (S, D), BF16, kind="ExternalInput").ap()
        xq = nc.dram_tensor("xq", (QL, D), BF16, kind="ExternalInput").ap()
        wq = nc.dram_tensor("wq", (D, D), BF16, kind="ExternalInput").ap()
        wk = nc.dram_tensor("wk", (D, D), BF16, kind="ExternalInput").ap()
        wv = nc.dram_tensor("wv", (D, D), BF16, kind="ExternalInput").ap()
        mask = nc.dram_tensor("mask", (P, 256), F32, kind="ExternalInput").ap()
        out = nc.dram_tensor("out", (QL, D), F32, kind="ExternalOutput").ap()
        with tile.TileContext(nc) as tc:
            _body(tc, x, xq, wq, wk, wv, mask, out)
        nc.compile()
        _PROG = nc
    return _PROG


def _mask_np(h):
    r = np.arange(P)[:, None]
    c = np.arange(P)[None, :]
    tri = np.where(c <= r, 0.0, NEG).astype(np.float32)
    m = np.zeros((P, 256), np.float32)
    if h == 0:
        m[:, :P] = tri
        m[:, P:] = NEG
    else:
        m[:, P:] = tri
    return m


def _prep_shared(inputs):
    bf = ml_dtypes.bfloat16
    wq = np.asarray(inputs["wq"], np.float32)
    wk = np.asarray(inputs["wk"], np.float32)
    wv = np.asarray(inputs["wv"], np.float32)
    return {
        "wq": np.ascontiguousarray((wq / 32.0).astype(bf)),
        "wk": np.ascontiguousarray(wk.astype(bf)),
        "wv": np.ascontiguousarray(wv.astype(bf)),
    }


def _in_map_for_core(inputs, core, shared=None):
    b, h = core // 2, core % 2
    if shared is None:
        shared = _prep_shared(inputs)
    xb = np.ascontiguousarray(
        np.asarray(inputs["x"], np.float32)[b].astype(ml_dtypes.bfloat16))
    xqb = np.ascontiguousarray(xb.reshape(NQT, 2, P, D)[:, h].reshape(QL, D))
    return {"x": xb, "xq": xqb, "mask": _mask_np(h), **shared}


def _run(inputs, trace=False, tmpdir=None):
    nc = _get_prog()
    shared = _prep_shared(inputs)
    in_maps = [_in_map_for_core(inputs, c, shared) for c in range(NCORES)]
    try:
        res = run_bass_kernel_spmd(nc, in_maps, core_ids=list(range(NCORES)),
                                   trace=trace, tmpdir=tmpdir)
    except Exception:
        # first execution of a fresh NEFF occasionally trips a transient
        # device error on this stack; one retry has always succeeded
        res = run_bass_kernel_spmd(nc, in_maps, core_ids=list(range(NCORES)),
                                   trace=trace, tmpdir=tmpdir)
    outf = np.empty((B, S, D), np.float32)
    for core in range(NCORES):
        b, h = core // 2, core % 2
        o = np.asarray(res.results[core]["out"], np.float32)
        outf[b].reshape(NQT, 2, P, D)[:, h] = o.reshape(NQT, P, D)
    return outf, res


def kernel(x, wq, wk, wv):
    outf, _ = _run({"x": x, "wq": wq, "wk": wk, "wv": wv}, trace=False)
    return outf
